# revision 32
# baseline (speedup 1.0000x reference)
"""Trainium2 Bass kernel for nn_BottleneckBit (ResNet bottleneck with ternary-
quantized convs + BN + SiLU + residual).

Strategy:
- Data-parallel over batch: 64 images -> 8 cores x 8 images.
- All convs lowered to TensorEngine matmuls with channels on partitions:
    conv1 (1x1, 1024->256):  four DoubleRow e4m3 k-pairs — the full 1024-ch
                             contraction runs at fp8 rate (x quantization
                             error budgeted via simulation: rel 1.80e-2 of
                             the 2e-2 gate, validated bit-exact against HW
                             on the mixed variant)
    conv2 (3x3, 256->256):   DoubleRow fp8: both 128-channel halves contract in
                             one pass. 9 shifted-tap DR matmuls per output
                             tile, reading a zero-padded 16x16-per-image fp8
                             buffer with image-pair pixels interleaved
                             (n = r*28 + c*2 + i) so the shifted window is a
                             4D AP [p, half, r, colpair].
    conv3 (1x1, 256->1024):  2 K-tiles bf16, a3 folded into the weights and c3
                             folded into the residual x' = x + c3 on the host
                             (conv1 corrected exactly via c1' = c1 - a1*(t1@c3),
                             CIN == COUT).
- BN affines ride the ACT engine: silu(a*psum + c) is one scalar.activation
  with per-partition scale/bias APs reading PSUM directly — no DVE affine in
  the l1/l2 epilogue chains, and per-j ACTs are emitted as soon as their
  PSUM half is stopped so the chain after the last matmul is one 392-col ACT.
- Batch-pair-major dataflow: x/xq are bp-major in DRAM. Because conv1 needs
  only the small fp8 xq (0.2MB/bp), l1 of all 4 bps runs off the front of the
  input stream; the bf16 x (residual-only) streams behind it. Stages
      A=l1, B=l2 taps j0 (+ACT j0), C=l2 taps j1 (+ACT j1),
      D=l3 units j0-3, E=l3 units j4-7
  are software-pipelined A0 A1 B0 C0 B1 C1 A2 D0 E0 B2 C2 A3 D1 E1 B3 C3
  D2 E2 D3 E3 — every producer->consumer epilogue hides under >=1
  intervening stage of PE work and the out stream starts ~1/3 in.
- Single sync-queue DMA with issue-order = priority order (each dma_start
  costs ~0.7us of issue time on its engine, so queue assignment matters):
  w1, xq0, w2, xq1, xq2, xq3, w3, then the residual x halves; out DMAs are
  also on sync (it is idle mid-kernel; they must not ride the busy ACT
  queue). Only cc rides gpsimd.
- l3 epilogue: residual + silu via one pair-batched DVE tensor_add
  psum->stage (the DVE is the psum tile's last reader, so psum recycling
  never waits on the busier ACT queue) + a quad pure-silu ACT for bp0..2;
  the bp3 tail keeps the same 'stage' mode but with per-unit pair ACTs and
  pair DMAs so the final outputs stream out as soon as each j-pair is done.
- Ternary weight trick: wq = clip(round(w/s),-1,1)*s. The {-1,0,1} ternary part
  is exact in fp8 (and the e4m3 x upcasts exactly through the DR e6m3 path);
  per-out-channel scale s and BN fold into (a, c).
- h1 is stored e4m3 with the image-pair/channel-half pixels interleaved
  (offset r*64 + c*4 + i*2 + e) so the DoubleRow moving operand reads its fp8
  pair in one 16-bit access — without this DR matmuls run at half speed.
- A short dummy-matmul block covers the first xq chunk's DMA lead-in and
  starts the HAM clock-gate warmup; mid-kernel PE gaps are all <1.5us so no
  keep-warm matmuls are needed (HAM re-throttles only after ~3.4us idle).
"""
import numpy as np
import ml_dtypes

import concourse.bass as bass
import concourse.mybir as mybir
from concourse import bacc
from concourse.tile import TileContext
from concourse.bass_utils import run_bass_kernel_spmd
from concourse.masks import make_identity


BN_EPS = 1e-5
Q_EPS = 1e-8

# Problem shape (hardcoded per contract)
B, CIN, H, W = 64, 1024, 14, 14
WIDTH, COUT = 256, 1024
N_CORES = 8
BC = B // N_CORES          # images per core = 8
PIX = H * W                # 196
P = 128
CIN_T = CIN // P           # 8
W_T = WIDTH // P           # 2
COUT_T = COUT // P         # 8
IPG = 2                    # images per matmul group (pixel-interleaved)
BP = BC // IPG             # 4 image-pair groups per core
NN = IPG * PIX             # 392 columns per matmul
HP, WP = H + 2, W + 2      # 16x16 padded image for the 3x3 conv
KP1 = CIN_T // 2           # 4 DoubleRow k-pairs for conv1
XB = CIN_T * NN            # bf16 x columns per bp (3136)
XQ = KP1 * NN * 2          # fp8 xq columns per bp (3136)
NDUMMY = 85                # 64-col pre-warm matmuls (cover the DMA lead-in)

_F32 = mybir.dt.float32
_BF16 = mybir.dt.bfloat16
_FP8 = mybir.dt.float8e4
_AF = mybir.ActivationFunctionType
_ALU = mybir.AluOpType
_DR = mybir.MatmulPerfMode.DoubleRow


def build(act_func=None):
    """Build the per-core Bass program (SPMD: same program on all 8 cores)."""
    if act_func is None:
        act_func = _AF.Silu
    nc = bacc.Bacc()

    # bp-major bf16 x (residual only): xd[p, bp*XB + k*NN + n], n = (r*W+c)*2+i
    xd = nc.declare_dram_parameter("x", [P, BP * XB], _BF16, isOutput=False)
    # bp-major e4m3 x for conv1's DoubleRow k-pairs:
    # xq[p, bp*XQ + kp*2*NN + n*2 + e], ch = (2*kp+e)*128+p
    xqd = nc.declare_dram_parameter("xq", [P, BP * XQ], _FP8, isOutput=False)
    w1d = nc.declare_dram_parameter("w1", [P, CIN_T * WIDTH], _FP8, isOutput=False)
    w2d = nc.declare_dram_parameter("w2", [P, W_T * 9 * WIDTH], _FP8, isOutput=False)
    w3d = nc.declare_dram_parameter("w3", [P, W_T * COUT], _BF16, isOutput=False)
    ccd = nc.declare_dram_parameter("cc", [P, 8 + COUT_T], _F32, isOutput=False)
    # output stays in the partition-folded layout [p, j*BC*PIX + n]; host unfolds
    outd = nc.declare_dram_parameter("out", [P, COUT_T * BC * PIX], _BF16, isOutput=True)

    with TileContext(nc) as tc:
        with tc.tile_pool(name="weights", bufs=1) as wpool, \
             tc.tile_pool(name="acts", bufs=1) as apool, \
             tc.tile_pool(name="outs", bufs=4) as opool, \
             tc.tile_pool(name="stage", bufs=3) as stpool, \
             tc.tile_pool(name="psum", bufs=2, space="PSUM") as pspool:

            # ---- one sync DMA queue, issue order = priority order (an issue
            # costs ~0.7us of engine time). conv1 needs only w1+xq, so l1 of
            # all bps runs off the front of the stream; bf16 x (residual) and
            # later weights ride behind. Out DMAs reuse this queue from ~20us
            # (it is idle by then). cc rides gpsimd in parallel. ----
            w1t = wpool.tile([P, CIN_T * WIDTH], _FP8, name="w1t")
            xt = apool.tile([P, BP * XB], _BF16, name="xt")
            xqt = apool.tile([P, BP * XQ], _FP8, name="xqt")

            def dma_xq(bp):
                a, b = bp * XQ, (bp + 1) * XQ
                nc.sync.dma_start(out=xqt[:, a:b], in_=xqd[:, a:b])

            def dma_x(bp, k0, k1):
                a, b = bp * XB + k0 * NN, bp * XB + k1 * NN
                nc.sync.dma_start(out=xt[:, a:b], in_=xd[:, a:b])

            # xq0 first (smallest critical chunk -> earliest completion sem),
            # then w1 in two halves so l1's kp0/kp1 matmuls can fire while
            # kp2/kp3 weights are still in flight
            dma_xq(0)
            nc.sync.dma_start(out=w1t[:, 0:4 * WIDTH], in_=w1d[:, 0:4 * WIDTH])
            nc.sync.dma_start(out=w1t[:, 4 * WIDTH:], in_=w1d[:, 4 * WIDTH:])
            dma_xq(1)
            w2t = wpool.tile([P, W_T * 9 * WIDTH], _FP8, name="w2t")
            nc.sync.dma_start(out=w2t[:, :], in_=w2d[:, :])
            dma_xq(2)
            dma_xq(3)
            w3t = wpool.tile([P, W_T * COUT], _BF16, name="w3t")
            nc.sync.dma_start(out=w3t[:, :], in_=w3d[:, :])
            dma_x(0, 0, 4)
            dma_x(0, 4, 8)
            dma_x(1, 0, 4)
            dma_x(1, 4, 8)
            dma_x(2, 0, 4)
            dma_x(2, 4, 8)
            dma_x(3, 0, 4)
            dma_x(3, 4, 8)
            cct = wpool.tile([P, 8 + COUT_T], _F32, name="cct")
            nc.gpsimd.dma_start(out=cct[:, :], in_=ccd[:, :])

            def xs(bp, t):          # x slice [128, NN] for (bpair, channel tile)
                return xt[:, bp * XB + t * NN: bp * XB + (t + 1) * NN]

            # ---- PE clock pre-warm: HAM needs ~3.4us of sustained PE activity
            # to lift the 1.2->2.4GHz clock gate; dummy matmuls bridge the
            # first xq chunk's DMA lead-in ----
            wsrc = apool.tile([P, 128], _BF16, name="wsrc")
            nc.vector.memset(wsrc[:, :], 0.0)
            identt = wpool.tile([P, P], _BF16, name="identt")
            make_identity(nc, identt[:, :])
            wps = pspool.tile([P, 1024], _F32, name="wps", tag="ps")
            for _ in range(NDUMMY):
                nc.tensor.matmul(wps[0:64, 0:64], wsrc[:, 0:64],
                                 wsrc[:, 0:64], start=True, stop=True)

            # ---- padded h1 buffers, fp8. Layout per bp group:
            # offset = r*64 + c*4 + i*2 + e  (e = channel half INNERMOST so
            # the DoubleRow moving operand reads its fp8 pair in one 16-bit
            # access) ----
            h1p = []
            for bp in range(BP):
                t = apool.tile([P, HP * WP * IPG * W_T], _FP8, name=f"h1p{bp}")
                nc.vector.memset(t[:, :], 0.0)
                h1p.append(t)
            h2 = [apool.tile([P, W_T * NN], _BF16, name=f"h2_{bp}")
                  for bp in range(BP)]

            w1v = w1t.rearrange("p (k m) -> p k m", k=CIN_T)
            xqv = xqt.rearrange("p (bp kp n e) -> p bp kp e n", bp=BP, kp=KP1,
                                e=2)
            w2v = w2t.rearrange("p (j t e m) -> p j t e m", j=W_T, t=9, e=W_T)

            # ---- stage A: layer 1 for one (bp, j) half. 1x1 conv 1024->256
            # as four DoubleRow e4m3 k-pairs into the half's OWN single-bank
            # psum tile (a shared 2-bank tile would let the framework
            # serialize j1's matmuls behind j0's ACT read), then one ACT
            # silu(a1*psum + c1) straight from PSUM into the padded fp8 h1 ----
            def l1_half(bp, j):
                ps1 = pspool.tile([P, 512], _F32, name=f"ps1_{bp}{j}",
                                  tag="ps1")
                for kp in range(KP1):
                    nc.tensor.matmul(
                        ps1[:, 0:NN],
                        w1v[:, 2 * kp:2 * kp + 2, j * P:(j + 1) * P],
                        xqv[:, bp, kp, :, :],
                        start=(kp == 0), stop=(kp == KP1 - 1),
                        perf_mode=_DR)
                src = ps1[:, 0:NN].rearrange("p (r c i) -> p r c i", r=H, c=W)
                dst = h1p[bp].rearrange(
                    "p (r c i e) -> p r c i e", r=HP, c=WP,
                    i=IPG)[:, 1:1 + H, 1:1 + W, :, j]
                nc.scalar.activation(dst, src, act_func,
                                     bias=cct[:, 2 + j:3 + j],
                                     scale=cct[:, 0 + j:1 + j])

            def l1_stage(bp):
                l1_half(bp, 0)
                l1_half(bp, 1)

            # ---- stages B/C: layer 2, 3x3 conv 256->256 via 9 shifted-tap
            # DoubleRow fp8 matmuls per output j-tile (B = j0, C = j1), each
            # j into its own single-bank psum tile. The per-j ACT
            # silu(a2*psum + c2) psum->h2 follows its own taps, so it
            # overlaps the next block's matmuls ----
            def l2_stage(bp, j):
                ps2 = pspool.tile([P, 512], _F32, name=f"ps2_{bp}{j}",
                                  tag="ps2")
                for tap in range(9):
                    dy, dx = divmod(tap, 3)
                    rhs = h1p[bp].rearrange(
                        "p (r ci e) -> p e r ci", r=HP, e=W_T
                    )[:, :, dy:dy + H, IPG * dx:IPG * dx + IPG * W]
                    nc.tensor.matmul(
                        ps2[:, 0:NN],
                        w2v[:, j, tap], rhs,
                        start=(tap == 0), stop=(tap == 8), perf_mode=_DR)
                nc.scalar.activation(h2[bp][:, j * NN:(j + 1) * NN],
                                     ps2[:, 0:NN],
                                     act_func, bias=cct[:, 6 + j:7 + j],
                                     scale=cct[:, 4 + j:5 + j])

            # ---- stages D/E: layer 3, 1x1 conv 256->1024 bf16 (a3 folded into
            # weights). One "unit" = a j-pair: 4 conv matmuls + epilogue.
            # The residual x already carries c3 (folded on the host), so the
            # epilogue per j-pair is either ONE pair-batched DVE add
            # psum->stage + a quad pure-silu ACT (mode 'stage' — no PE work),
            # or an identity matmul on the PE with a pair ACT straight from
            # PSUM (mode 'pe' — used in the bp3 tail where the PE would idle).
            # dma='pair' flushes half-size DMAs to shorten the tail. ----
            def l3_units(bp, modes='ssss', dma='quad', dma_eng=None):
                state = {}

                def mk(j0, mode):
                    def emit():
                        if mode == 'stage' and j0 % 4 == 0:
                            state['st'] = stpool.tile([P, 4 * NN], _F32,
                                                      name="st", tag="st")
                        if j0 % 4 == 0:
                            state['ot'] = opool.tile([P, 4 * NN], _BF16,
                                                     name="ot", tag="ot")
                        ot = state['ot']
                        ps3 = pspool.tile([P, 1024], _F32, name="ps3", tag="ps")
                        for dj in range(2):
                            j = j0 + dj
                            sl = ps3[:, dj * 512: dj * 512 + NN]
                            for k in range(W_T):
                                nc.tensor.matmul(
                                    sl,
                                    w3t[:, k * COUT + j * P:
                                        k * COUT + (j + 1) * P],
                                    h2[bp][:, k * NN:(k + 1) * NN],
                                    start=(k == 0),
                                    stop=(mode == 'stage' and k == W_T - 1))
                            if mode == 'pe':
                                nc.tensor.matmul(sl, identt[:, :], xs(bp, j),
                                                 start=False, stop=True)
                        pspair = ps3.rearrange("p (g n) -> p g n", g=2)[
                            :, :, 0:NN]
                        otpair = ot.rearrange("p (g n) -> p g n", g=4)[
                            :, (j0 % 4):(j0 % 4) + 2, :]
                        if mode == 'stage':
                            xpair = xt.rearrange(
                                "p (bp t n) -> p bp t n", bp=BP, t=CIN_T)[
                                :, bp, j0:j0 + 2, :]
                            stpair = state['st'][
                                :, (j0 % 4) * NN:(j0 % 4 + 2) * NN
                            ].rearrange("p (g n) -> p g n", g=2)
                            nc.vector.tensor_add(out=stpair, in0=pspair,
                                                 in1=xpair)
                            if dma == 'pair':
                                nc.scalar.activation(otpair, stpair, act_func)
                            elif j0 % 4 == 2:
                                nc.scalar.activation(ot[:, :],
                                                     state['st'][:, :],
                                                     act_func)
                        else:
                            nc.scalar.activation(otpair, pspair, act_func)
                        if dma == 'pair' or j0 % 4 == 2:
                            jlo = j0 if dma == 'pair' else j0 - 2
                            nj = 2 if dma == 'pair' else 4
                            dmadst = outd.rearrange(
                                "p (j n) -> p j n", j=COUT_T)[
                                :, jlo:jlo + nj, bp * NN:(bp + 1) * NN]
                            src = ot.rearrange("p (g n) -> p g n", g=4)[
                                :, (jlo % 4):(jlo % 4) + nj, :]
                            (dma_eng or nc.sync).dma_start(out=dmadst, in_=src)
                    return emit
                return [mk(j0, {'s': 'stage', 'p': 'pe'}[m])
                        for j0, m in zip((0, 2, 4, 6), modes)]

            # ---- software-pipelined emission across the 4 bps ----
            units = {bp: l3_units(bp, modes='ssss') for bp in range(3)}
            units[3] = l3_units(3, modes='ssss', dma='pair')

            def D(bp):
                units[bp][0]()
                units[bp][1]()

            def E(bp):
                units[bp][2]()
                units[bp][3]()

            l1_stage(0)
            l1_stage(1)
            l2_stage(0, 0)
            l2_stage(0, 1)
            l2_stage(1, 0)
            l2_stage(1, 1)
            l1_stage(2)
            D(0)
            l2_stage(2, 0)
            E(0)
            l2_stage(2, 1)
            l1_half(3, 0)
            D(1)
            l1_half(3, 1)
            E(1)
            l2_stage(3, 0)
            l2_stage(3, 1)
            D(2)
            D(3)
            E(2)
            E(3)

    nc.finalize()
    return nc


def _prep_host(x, w1, b1, g1, be1, m1, v1,
               w2, b2, g2, be2, m2, v2,
               w3, b3, g3, be3, m3, v3):
    """Quantize weights, fold BN, and lay out device arrays."""
    def quant(w):
        w = np.asarray(w, np.float32)
        s = np.median(np.abs(w).reshape(w.shape[0], -1), axis=1)
        s = np.maximum(s, np.float32(Q_EPS)).astype(np.float32)
        t = np.clip(np.round(w / s[:, None, None, None]), -1.0, 1.0).astype(np.float32)
        return t, s

    def fold(s, b, g, be, m, v):
        sc = np.asarray(g, np.float64) / np.sqrt(np.asarray(v, np.float64) + BN_EPS)
        a = (np.asarray(s, np.float64) * sc).astype(np.float32)
        c = (np.asarray(b, np.float64) * sc + np.asarray(be, np.float64)
             - np.asarray(m, np.float64) * sc).astype(np.float32)
        return a, c

    t1, s1 = quant(w1)
    t2, s2 = quant(w2)
    t3, s3 = quant(w3)
    a1, c1 = fold(s1, b1, g1, be1, m1, v1)
    a2, c2 = fold(s2, b2, g2, be2, m2, v2)
    a3, c3 = fold(s3, b3, g3, be3, m3, v3)

    # Fold c3 into the residual input: ship x' = x + c3 (broadcast per
    # channel; CIN == COUT so the same tensor serves conv1 and the residual).
    # conv1 then sees a per-channel constant shift, corrected exactly in c1:
    # conv1(x + c3) = conv1(x) + t1 @ c3  =>  c1 -= a1 * (t1 @ c3)
    t1c3 = t1[:, :, 0, 0].astype(np.float64) @ c3.astype(np.float64)
    c1 = (c1.astype(np.float64) - a1.astype(np.float64) * t1c3).astype(np.float32)

    bf = ml_dtypes.bfloat16
    fp8 = ml_dtypes.float8_e4m3

    def part_fold(m2d):
        # [K, M] -> [128, (K//128)*M]: row k*128+p lands at [p, k*M+m]
        kk, mm = m2d.shape
        return np.ascontiguousarray(
            m2d.reshape(kk // P, P, mm).transpose(1, 0, 2).reshape(P, -1))

    w1_dev = part_fold(t1[:, :, 0, 0].T).astype(fp8)
    # w2_dev[p, j, tap, e, m] = t2[j*128+m, e*128+p, dy, dx]
    w2_dev = np.ascontiguousarray(
        t2.reshape(W_T, P, W_T, P, 3, 3)           # j, m, e, p, dy, dx
        .transpose(3, 0, 4, 5, 2, 1)               # p, j, dy, dx, e, m
        .reshape(P, W_T * 9 * W_T * P)).astype(fp8)
    w3_dev = part_fold((t3[:, :, 0, 0] * a3[:, None]).T).astype(bf)

    cc = np.zeros((P, 8 + COUT_T), np.float32)
    cc[:, 0:2] = a1.reshape(W_T, P).T
    cc[:, 2:4] = c1.reshape(W_T, P).T
    cc[:, 4:6] = a2.reshape(W_T, P).T
    cc[:, 6:8] = c2.reshape(W_T, P).T
    cc[:, 8:] = c3.reshape(COUT_T, P).T

    const = {"w1": w1_dev, "w2": w2_dev, "w3": w3_dev,
             "cc": np.ascontiguousarray(cc)}

    x = np.asarray(x, np.float32) + c3[None, :, None, None]
    in_maps = []
    for c in range(N_CORES):
        # xd[p, bp*XB + k*NN + (r*W+cw)*2 + i] = x[c*BC+bp*2+i, k*128+p, r, cw]
        xb = x[c * BC:(c + 1) * BC].reshape(BP, IPG, CIN_T, P, H, W)
        xc = np.ascontiguousarray(
            xb.transpose(3, 0, 2, 4, 5, 1).reshape(P, BP * XB))
        # xq[p, bp*XQ + kp*2*NN + ((r*W+cw)*2+i)*2 + e] = x'[ch=(2*kp+e)*128+p]
        xq = xb.reshape(BP, IPG, KP1, 2, P, H, W)
        xq = np.ascontiguousarray(
            xq.transpose(4, 0, 2, 5, 6, 1, 3).reshape(P, BP * XQ))
        in_maps.append({"x": xc.astype(bf), "xq": xq.astype(fp8), **const})
    return in_maps


def _run(inputs, trace=False, act_func=None, **spmd_kwargs):
    nc = build(act_func)
    in_maps = _prep_host(**inputs)
    res = run_bass_kernel_spmd(nc, in_maps, list(range(N_CORES)),
                               trace=trace, **spmd_kwargs)
    outs = []
    for c in range(N_CORES):
        of = res.results[c]["out"].astype(np.float32)    # folded [P, COUT_T*BC*PIX]
        oc = of.reshape(P, COUT_T, BP, H, W, IPG)
        oc = oc.transpose(2, 5, 1, 0, 3, 4).reshape(BC, COUT, H, W)
        outs.append(oc)
    full = np.concatenate(outs, axis=0).astype(np.float32)
    return full, res


def kernel(**inputs):
    out, _ = _run(inputs)
    return out


# revision 33
# speedup vs baseline: 1.0365x; 1.0365x over previous
"""Trainium2 Bass kernel for nn_BottleneckBit (ResNet bottleneck with ternary-
quantized convs + BN + SiLU + residual).

Strategy:
- Data-parallel over batch: 64 images -> 8 cores x 8 images.
- All convs lowered to TensorEngine matmuls with channels on partitions:
    conv1 (1x1, 1024->256):  four DoubleRow e4m3 k-pairs — the full 1024-ch
                             contraction runs at fp8 rate (x quantization
                             error budgeted via simulation: rel 1.80e-2 of
                             the 2e-2 gate, validated bit-exact against HW
                             on the mixed variant)
    conv2 (3x3, 256->256):   DoubleRow fp8: both 128-channel halves contract in
                             one pass. 9 shifted-tap DR matmuls per output
                             tile, reading a zero-padded 16x16-per-image fp8
                             buffer with image-pair pixels interleaved
                             (n = r*28 + c*2 + i) so the shifted window is a
                             4D AP [p, half, r, colpair].
    conv3 (1x1, 256->1024):  2 K-tiles bf16, a3 folded into the weights and c3
                             folded into the residual x' = x + c3 on the host
                             (conv1 corrected exactly via c1' = c1 - a1*(t1@c3),
                             CIN == COUT).
- BN affines ride the ACT engine: silu(a*psum + c) is one scalar.activation
  with per-partition scale/bias APs reading PSUM directly — no DVE affine in
  the l1/l2 epilogue chains, and per-j ACTs are emitted as soon as their
  PSUM half is stopped so the chain after the last matmul is one 392-col ACT.
- Batch-pair-major dataflow: x/xq are bp-major in DRAM. Because conv1 needs
  only the small fp8 xq (0.2MB/bp), l1 of all 4 bps runs off the front of the
  input stream; the bf16 x (residual-only) streams behind it. Stages
      A=l1, B=l2 taps j0 (+ACT j0), C=l2 taps j1 (+ACT j1),
      D=l3 units j0-3, E=l3 units j4-7
  are software-pipelined A0 A1 B0 C0 B1 C1 A2 D0 E0 B2 C2 A3 D1 E1 B3 C3
  D2 E2 D3 E3 — every producer->consumer epilogue hides under >=1
  intervening stage of PE work and the out stream starts ~1/3 in.
- Single sync-queue DMA with issue-order = priority order (each dma_start
  costs ~0.7us of issue time on its engine, so queue assignment matters):
  w1, xq0, w2, xq1, xq2, xq3, w3, then the residual x halves; out DMAs are
  also on sync (it is idle mid-kernel; they must not ride the busy ACT
  queue). Only cc rides gpsimd.
- l3 epilogue: residual + silu via one pair-batched DVE tensor_add
  psum->stage (the DVE is the psum tile's last reader, so psum recycling
  never waits on the busier ACT queue) + a quad pure-silu ACT for bp0..2;
  the bp3 tail keeps the same 'stage' mode but with per-unit pair ACTs and
  pair DMAs so the final outputs stream out as soon as each j-pair is done.
- Ternary weight trick: wq = clip(round(w/s),-1,1)*s. The {-1,0,1} ternary part
  is exact in fp8 (and the e4m3 x upcasts exactly through the DR e6m3 path);
  per-out-channel scale s and BN fold into (a, c).
- h1 is stored e4m3 with the image-pair/channel-half pixels interleaved
  (offset r*64 + c*4 + i*2 + e) so the DoubleRow moving operand reads its fp8
  pair in one 16-bit access — without this DR matmuls run at half speed.
- A short dummy-matmul block covers the first xq chunk's DMA lead-in and
  starts the HAM clock-gate warmup; mid-kernel PE gaps are all <1.5us so no
  keep-warm matmuls are needed (HAM re-throttles only after ~3.4us idle).
"""
import numpy as np
import ml_dtypes

import concourse.bass as bass
import concourse.mybir as mybir
from concourse import bacc
from concourse.tile import TileContext
from concourse.bass_utils import run_bass_kernel_spmd
from concourse.masks import make_identity


BN_EPS = 1e-5
Q_EPS = 1e-8

# Problem shape (hardcoded per contract)
B, CIN, H, W = 64, 1024, 14, 14
WIDTH, COUT = 256, 1024
N_CORES = 8
BC = B // N_CORES          # images per core = 8
PIX = H * W                # 196
P = 128
CIN_T = CIN // P           # 8
W_T = WIDTH // P           # 2
COUT_T = COUT // P         # 8
IPG = 2                    # images per matmul group (pixel-interleaved)
BP = BC // IPG             # 4 image-pair groups per core
NN = IPG * PIX             # 392 columns per matmul
HP, WP = H + 2, W + 2      # 16x16 padded image for the 3x3 conv
KP1 = CIN_T // 2           # 4 DoubleRow k-pairs for conv1
XB = CIN_T * NN            # bf16 x columns per bp (3136)
XQ = KP1 * NN * 2          # fp8 xq columns per bp (3136)
NDUMMY = 85                # 64-col pre-warm matmuls (cover the DMA lead-in)

_F32 = mybir.dt.float32
_BF16 = mybir.dt.bfloat16
_FP8 = mybir.dt.float8e4
_AF = mybir.ActivationFunctionType
_ALU = mybir.AluOpType
_DR = mybir.MatmulPerfMode.DoubleRow


def build(act_func=None):
    """Build the per-core Bass program (SPMD: same program on all 8 cores)."""
    if act_func is None:
        act_func = _AF.Silu
    nc = bacc.Bacc()

    # bp-major bf16 x (residual only): xd[p, bp*XB + k*NN + n], n = (r*W+c)*2+i
    xd = nc.declare_dram_parameter("x", [P, BP * XB], _BF16, isOutput=False)
    # bp-major e4m3 x for conv1's DoubleRow k-pairs:
    # xq[p, bp*XQ + kp*2*NN + n*2 + e], ch = (2*kp+e)*128+p
    xqd = nc.declare_dram_parameter("xq", [P, BP * XQ], _FP8, isOutput=False)
    w1d = nc.declare_dram_parameter("w1", [P, CIN_T * WIDTH], _FP8, isOutput=False)
    w2d = nc.declare_dram_parameter("w2", [P, W_T * 9 * WIDTH], _FP8, isOutput=False)
    w3d = nc.declare_dram_parameter("w3", [P, W_T * COUT], _BF16, isOutput=False)
    ccd = nc.declare_dram_parameter("cc", [P, 8 + COUT_T], _F32, isOutput=False)
    # output stays in the partition-folded layout [p, j*BC*PIX + n]; host unfolds
    outd = nc.declare_dram_parameter("out", [P, COUT_T * BC * PIX], _BF16, isOutput=True)

    with TileContext(nc) as tc:
        with tc.tile_pool(name="weights", bufs=1) as wpool, \
             tc.tile_pool(name="acts", bufs=1) as apool, \
             tc.tile_pool(name="outs", bufs=4) as opool, \
             tc.tile_pool(name="stage", bufs=3) as stpool, \
             tc.tile_pool(name="psum", bufs=2, space="PSUM") as pspool:

            # ---- one sync DMA queue, issue order = priority order (an issue
            # costs ~0.7us of engine time). conv1 needs only w1+xq, so l1 of
            # all bps runs off the front of the stream; bf16 x (residual) and
            # later weights ride behind. Out DMAs reuse this queue from ~20us
            # (it is idle by then). cc rides gpsimd in parallel. ----
            w1t = wpool.tile([P, CIN_T * WIDTH], _FP8, name="w1t")
            xt = apool.tile([P, BP * XB], _BF16, name="xt")
            xqt = apool.tile([P, BP * XQ], _FP8, name="xqt")

            def dma_xq(bp):
                a, b = bp * XQ, (bp + 1) * XQ
                nc.sync.dma_start(out=xqt[:, a:b], in_=xqd[:, a:b])

            def dma_x(bp, k0, k1):
                a, b = bp * XB + k0 * NN, bp * XB + k1 * NN
                nc.sync.dma_start(out=xt[:, a:b], in_=xd[:, a:b])

            # xq0 first (smallest critical chunk -> earliest completion sem),
            # then w1 in two halves so l1's kp0/kp1 matmuls can fire while
            # kp2/kp3 weights are still in flight
            dma_xq(0)
            nc.sync.dma_start(out=w1t[:, 0:4 * WIDTH], in_=w1d[:, 0:4 * WIDTH])
            nc.sync.dma_start(out=w1t[:, 4 * WIDTH:], in_=w1d[:, 4 * WIDTH:])
            dma_xq(1)
            w2t = wpool.tile([P, W_T * 9 * WIDTH], _FP8, name="w2t")
            nc.sync.dma_start(out=w2t[:, :], in_=w2d[:, :])
            dma_xq(2)
            dma_xq(3)
            w3t = wpool.tile([P, W_T * COUT], _BF16, name="w3t")
            nc.sync.dma_start(out=w3t[:, :], in_=w3d[:, :])
            dma_x(0, 0, 4)
            dma_x(0, 4, 8)
            dma_x(1, 0, 4)
            dma_x(1, 4, 8)
            dma_x(2, 0, 4)
            dma_x(2, 4, 8)
            dma_x(3, 0, 4)
            dma_x(3, 4, 8)
            cct = wpool.tile([P, 8 + COUT_T], _F32, name="cct")
            nc.gpsimd.dma_start(out=cct[:, :], in_=ccd[:, :])

            def xs(bp, t):          # x slice [128, NN] for (bpair, channel tile)
                return xt[:, bp * XB + t * NN: bp * XB + (t + 1) * NN]

            # ---- PE clock pre-warm: HAM needs ~3.4us of sustained PE activity
            # to lift the 1.2->2.4GHz clock gate; dummy matmuls bridge the
            # first xq chunk's DMA lead-in ----
            wsrc = apool.tile([P, 128], _BF16, name="wsrc")
            nc.vector.memset(wsrc[:, :], 0.0)
            identt = wpool.tile([P, P], _BF16, name="identt")
            make_identity(nc, identt[:, :])
            wps = pspool.tile([P, 1024], _F32, name="wps", tag="ps")
            for _ in range(NDUMMY):
                nc.tensor.matmul(wps[0:64, 0:64], wsrc[:, 0:64],
                                 wsrc[:, 0:64], start=True, stop=True)

            # ---- padded h1 buffers, fp8. Layout per bp group:
            # offset = r*64 + c*4 + i*2 + e  (e = channel half INNERMOST so
            # the DoubleRow moving operand reads its fp8 pair in one 16-bit
            # access) ----
            h1p = []
            for bp in range(BP):
                t = apool.tile([P, HP * WP * IPG * W_T], _FP8, name=f"h1p{bp}")
                nc.vector.memset(t[:, :], 0.0)
                h1p.append(t)
            h2 = [apool.tile([P, W_T * NN], _BF16, name=f"h2_{bp}")
                  for bp in range(BP)]

            w1v = w1t.rearrange("p (k m) -> p k m", k=CIN_T)
            xqv = xqt.rearrange("p (bp kp n e) -> p bp kp e n", bp=BP, kp=KP1,
                                e=2)
            w2v = w2t.rearrange("p (j t e m) -> p j t e m", j=W_T, t=9, e=W_T)

            # ---- stage A: layer 1 for one (bp, j) half. 1x1 conv 1024->256
            # as four DoubleRow e4m3 k-pairs into the half's OWN single-bank
            # psum tile (a shared 2-bank tile would let the framework
            # serialize j1's matmuls behind j0's ACT read), then one ACT
            # silu(a1*psum + c1) straight from PSUM into the padded fp8 h1 ----
            def l1_half(bp, j):
                ps1 = pspool.tile([P, 512], _F32, name=f"ps1_{bp}{j}",
                                  tag="ps1")
                for kp in range(KP1):
                    nc.tensor.matmul(
                        ps1[:, 0:NN],
                        w1v[:, 2 * kp:2 * kp + 2, j * P:(j + 1) * P],
                        xqv[:, bp, kp, :, :],
                        start=(kp == 0), stop=(kp == KP1 - 1),
                        perf_mode=_DR)
                src = ps1[:, 0:NN].rearrange("p (r c i) -> p r c i", r=H, c=W)
                dst = h1p[bp].rearrange(
                    "p (r c i e) -> p r c i e", r=HP, c=WP,
                    i=IPG)[:, 1:1 + H, 1:1 + W, :, j]
                nc.scalar.activation(dst, src, act_func,
                                     bias=cct[:, 2 + j:3 + j],
                                     scale=cct[:, 0 + j:1 + j])

            def l1_stage(bp):
                l1_half(bp, 0)
                l1_half(bp, 1)

            # ---- stages B/C: layer 2, 3x3 conv 256->256 via 9 shifted-tap
            # DoubleRow fp8 matmuls per output j-tile (B = j0, C = j1), each
            # j into its own single-bank psum tile. The per-j ACT
            # silu(a2*psum + c2) psum->h2 follows its own taps, so it
            # overlaps the next block's matmuls ----
            def l2_stage(bp, j):
                ps2 = pspool.tile([P, 512], _F32, name=f"ps2_{bp}{j}",
                                  tag="ps2")
                for tap in range(9):
                    dy, dx = divmod(tap, 3)
                    rhs = h1p[bp].rearrange(
                        "p (r ci e) -> p e r ci", r=HP, e=W_T
                    )[:, :, dy:dy + H, IPG * dx:IPG * dx + IPG * W]
                    nc.tensor.matmul(
                        ps2[:, 0:NN],
                        w2v[:, j, tap], rhs,
                        start=(tap == 0), stop=(tap == 8), perf_mode=_DR)
                nc.scalar.activation(h2[bp][:, j * NN:(j + 1) * NN],
                                     ps2[:, 0:NN],
                                     act_func, bias=cct[:, 6 + j:7 + j],
                                     scale=cct[:, 4 + j:5 + j])

            # ---- stages D/E: layer 3, 1x1 conv 256->1024 bf16 (a3 folded into
            # weights). One "unit" = a j-pair: 4 conv matmuls + epilogue.
            # The residual x already carries c3 (folded on the host), so the
            # epilogue per j-pair is either ONE pair-batched DVE add
            # psum->stage + a quad pure-silu ACT (mode 'stage' — no PE work),
            # or an identity matmul on the PE with a pair ACT straight from
            # PSUM (mode 'pe' — used in the bp3 tail where the PE would idle).
            # dma='pair' flushes half-size DMAs to shorten the tail. ----
            def l3_units(bp, modes='ssss', dma='quad', dma_eng=None):
                state = {}

                def mk(j0, mode):
                    def emit():
                        if mode == 'stage' and j0 % 4 == 0:
                            state['st'] = stpool.tile([P, 4 * NN], _F32,
                                                      name="st", tag="st")
                        if j0 % 4 == 0:
                            state['ot'] = opool.tile([P, 4 * NN], _BF16,
                                                     name="ot", tag="ot")
                        ot = state['ot']
                        ps3 = pspool.tile([P, 1024], _F32, name="ps3", tag="ps")
                        for dj in range(2):
                            j = j0 + dj
                            sl = ps3[:, dj * 512: dj * 512 + NN]
                            for k in range(W_T):
                                nc.tensor.matmul(
                                    sl,
                                    w3t[:, k * COUT + j * P:
                                        k * COUT + (j + 1) * P],
                                    h2[bp][:, k * NN:(k + 1) * NN],
                                    start=(k == 0),
                                    stop=(mode == 'stage' and k == W_T - 1))
                            if mode == 'pe':
                                nc.tensor.matmul(sl, identt[:, :], xs(bp, j),
                                                 start=False, stop=True)
                        pspair = ps3.rearrange("p (g n) -> p g n", g=2)[
                            :, :, 0:NN]
                        otpair = ot.rearrange("p (g n) -> p g n", g=4)[
                            :, (j0 % 4):(j0 % 4) + 2, :]
                        if mode == 'stage':
                            xpair = xt.rearrange(
                                "p (bp t n) -> p bp t n", bp=BP, t=CIN_T)[
                                :, bp, j0:j0 + 2, :]
                            stpair = state['st'][
                                :, (j0 % 4) * NN:(j0 % 4 + 2) * NN
                            ].rearrange("p (g n) -> p g n", g=2)
                            nc.vector.tensor_add(out=stpair, in0=pspair,
                                                 in1=xpair)
                            if dma == 'pair':
                                nc.scalar.activation(otpair, stpair, act_func)
                            elif j0 % 4 == 2:
                                nc.scalar.activation(ot[:, :],
                                                     state['st'][:, :],
                                                     act_func)
                        else:
                            nc.scalar.activation(otpair, pspair, act_func)
                        if dma == 'pair' or j0 % 4 == 2:
                            jlo = j0 if dma == 'pair' else j0 - 2
                            nj = 2 if dma == 'pair' else 4
                            dmadst = outd.rearrange(
                                "p (j n) -> p j n", j=COUT_T)[
                                :, jlo:jlo + nj, bp * NN:(bp + 1) * NN]
                            src = ot.rearrange("p (g n) -> p g n", g=4)[
                                :, (jlo % 4):(jlo % 4) + nj, :]
                            (dma_eng or nc.sync).dma_start(out=dmadst, in_=src)
                    return emit
                return [mk(j0, {'s': 'stage', 'p': 'pe'}[m])
                        for j0, m in zip((0, 2, 4, 6), modes)]

            # ---- software-pipelined emission across the 4 bps ----
            units = {bp: l3_units(bp, modes='ssss') for bp in range(3)}
            units[3] = l3_units(3, modes='ssss', dma='pair')

            def D(bp):
                units[bp][0]()
                units[bp][1]()

            def E(bp):
                units[bp][2]()
                units[bp][3]()

            l1_stage(0)
            l1_stage(1)
            l2_stage(0, 0)
            l2_stage(0, 1)
            l2_stage(1, 0)
            l2_stage(1, 1)
            l1_stage(2)
            D(0)
            l2_stage(2, 0)
            E(0)
            l2_stage(2, 1)
            l1_half(3, 0)
            D(1)
            l1_half(3, 1)
            E(1)
            l2_stage(3, 0)
            l2_stage(3, 1)
            D(2)
            E(2)
            D(3)
            E(3)

    nc.finalize()
    return nc


def _prep_host(x, w1, b1, g1, be1, m1, v1,
               w2, b2, g2, be2, m2, v2,
               w3, b3, g3, be3, m3, v3):
    """Quantize weights, fold BN, and lay out device arrays."""
    def quant(w):
        w = np.asarray(w, np.float32)
        s = np.median(np.abs(w).reshape(w.shape[0], -1), axis=1)
        s = np.maximum(s, np.float32(Q_EPS)).astype(np.float32)
        t = np.clip(np.round(w / s[:, None, None, None]), -1.0, 1.0).astype(np.float32)
        return t, s

    def fold(s, b, g, be, m, v):
        sc = np.asarray(g, np.float64) / np.sqrt(np.asarray(v, np.float64) + BN_EPS)
        a = (np.asarray(s, np.float64) * sc).astype(np.float32)
        c = (np.asarray(b, np.float64) * sc + np.asarray(be, np.float64)
             - np.asarray(m, np.float64) * sc).astype(np.float32)
        return a, c

    t1, s1 = quant(w1)
    t2, s2 = quant(w2)
    t3, s3 = quant(w3)
    a1, c1 = fold(s1, b1, g1, be1, m1, v1)
    a2, c2 = fold(s2, b2, g2, be2, m2, v2)
    a3, c3 = fold(s3, b3, g3, be3, m3, v3)

    # Fold c3 into the residual input: ship x' = x + c3 (broadcast per
    # channel; CIN == COUT so the same tensor serves conv1 and the residual).
    # conv1 then sees a per-channel constant shift, corrected exactly in c1:
    # conv1(x + c3) = conv1(x) + t1 @ c3  =>  c1 -= a1 * (t1 @ c3)
    t1c3 = t1[:, :, 0, 0].astype(np.float64) @ c3.astype(np.float64)
    c1 = (c1.astype(np.float64) - a1.astype(np.float64) * t1c3).astype(np.float32)

    bf = ml_dtypes.bfloat16
    fp8 = ml_dtypes.float8_e4m3

    def part_fold(m2d):
        # [K, M] -> [128, (K//128)*M]: row k*128+p lands at [p, k*M+m]
        kk, mm = m2d.shape
        return np.ascontiguousarray(
            m2d.reshape(kk // P, P, mm).transpose(1, 0, 2).reshape(P, -1))

    w1_dev = part_fold(t1[:, :, 0, 0].T).astype(fp8)
    # w2_dev[p, j, tap, e, m] = t2[j*128+m, e*128+p, dy, dx]
    w2_dev = np.ascontiguousarray(
        t2.reshape(W_T, P, W_T, P, 3, 3)           # j, m, e, p, dy, dx
        .transpose(3, 0, 4, 5, 2, 1)               # p, j, dy, dx, e, m
        .reshape(P, W_T * 9 * W_T * P)).astype(fp8)
    w3_dev = part_fold((t3[:, :, 0, 0] * a3[:, None]).T).astype(bf)

    cc = np.zeros((P, 8 + COUT_T), np.float32)
    cc[:, 0:2] = a1.reshape(W_T, P).T
    cc[:, 2:4] = c1.reshape(W_T, P).T
    cc[:, 4:6] = a2.reshape(W_T, P).T
    cc[:, 6:8] = c2.reshape(W_T, P).T
    cc[:, 8:] = c3.reshape(COUT_T, P).T

    const = {"w1": w1_dev, "w2": w2_dev, "w3": w3_dev,
             "cc": np.ascontiguousarray(cc)}

    x = np.asarray(x, np.float32) + c3[None, :, None, None]
    in_maps = []
    for c in range(N_CORES):
        # xd[p, bp*XB + k*NN + (r*W+cw)*2 + i] = x[c*BC+bp*2+i, k*128+p, r, cw]
        xb = x[c * BC:(c + 1) * BC].reshape(BP, IPG, CIN_T, P, H, W)
        xc = np.ascontiguousarray(
            xb.transpose(3, 0, 2, 4, 5, 1).reshape(P, BP * XB))
        # xq[p, bp*XQ + kp*2*NN + ((r*W+cw)*2+i)*2 + e] = x'[ch=(2*kp+e)*128+p]
        xq = xb.reshape(BP, IPG, KP1, 2, P, H, W)
        xq = np.ascontiguousarray(
            xq.transpose(4, 0, 2, 5, 6, 1, 3).reshape(P, BP * XQ))
        in_maps.append({"x": xc.astype(bf), "xq": xq.astype(fp8), **const})
    return in_maps


def _run(inputs, trace=False, act_func=None, **spmd_kwargs):
    nc = build(act_func)
    in_maps = _prep_host(**inputs)
    res = run_bass_kernel_spmd(nc, in_maps, list(range(N_CORES)),
                               trace=trace, **spmd_kwargs)
    outs = []
    for c in range(N_CORES):
        of = res.results[c]["out"].astype(np.float32)    # folded [P, COUT_T*BC*PIX]
        oc = of.reshape(P, COUT_T, BP, H, W, IPG)
        oc = oc.transpose(2, 5, 1, 0, 3, 4).reshape(BC, COUT, H, W)
        outs.append(oc)
    full = np.concatenate(outs, axis=0).astype(np.float32)
    return full, res


def kernel(**inputs):
    out, _ = _run(inputs)
    return out


# revision 34
# speedup vs baseline: 1.0543x; 1.0172x over previous
"""Trainium2 Bass kernel for nn_BottleneckBit (ResNet bottleneck with ternary-
quantized convs + BN + SiLU + residual).

Strategy:
- Data-parallel over batch: 64 images -> 8 cores x 8 images.
- All convs lowered to TensorEngine matmuls with channels on partitions:
    conv1 (1x1, 1024->256):  four DoubleRow e4m3 k-pairs — the full 1024-ch
                             contraction runs at fp8 rate (x quantization
                             error budgeted via simulation: rel 1.80e-2 of
                             the 2e-2 gate, validated bit-exact against HW
                             on the mixed variant)
    conv2 (3x3, 256->256):   DoubleRow fp8: both 128-channel halves contract in
                             one pass. 9 shifted-tap DR matmuls per output
                             tile, reading a zero-padded 16x16-per-image fp8
                             buffer with image-pair pixels interleaved
                             (n = r*28 + c*2 + i) so the shifted window is a
                             4D AP [p, half, r, colpair].
    conv3 (1x1, 256->1024):  2 K-tiles bf16, a3 folded into the weights and c3
                             folded into the residual x' = x + c3 on the host
                             (conv1 corrected exactly via c1' = c1 - a1*(t1@c3),
                             CIN == COUT).
- BN affines ride the ACT engine: silu(a*psum + c) is one scalar.activation
  with per-partition scale/bias APs reading PSUM directly — no DVE affine in
  the l1/l2 epilogue chains, and per-j ACTs are emitted as soon as their
  PSUM half is stopped so the chain after the last matmul is one 392-col ACT.
- Batch-pair-major dataflow: x/xq are bp-major in DRAM. Because conv1 needs
  only the small fp8 xq (0.2MB/bp), l1 of all 4 bps runs off the front of the
  input stream; the bf16 x (residual-only) streams behind it. Stages
      A=l1, B=l2 taps j0 (+ACT j0), C=l2 taps j1 (+ACT j1),
      D=l3 units j0-3, E=l3 units j4-7
  are software-pipelined A0 A1 B0 C0 B1 C1 A2 D0 E0 B2 C2 A3 D1 E1 B3 C3
  D2 E2 D3 E3 — every producer->consumer epilogue hides under >=1
  intervening stage of PE work and the out stream starts ~1/3 in.
- Single sync-queue DMA with issue-order = priority order (each dma_start
  costs ~0.7us of issue time on its engine, so queue assignment matters):
  w1, xq0, w2, xq1, xq2, xq3, w3, then the residual x halves; out DMAs are
  also on sync (it is idle mid-kernel; they must not ride the busy ACT
  queue). Only cc rides gpsimd.
- l3 epilogue: residual + silu via one pair-batched DVE tensor_add
  psum->stage (the DVE is the psum tile's last reader, so psum recycling
  never waits on the busier ACT queue) + a quad pure-silu ACT for bp0..2;
  the bp3 tail keeps the same 'stage' mode but with per-unit pair ACTs and
  pair DMAs so the final outputs stream out as soon as each j-pair is done.
- Ternary weight trick: wq = clip(round(w/s),-1,1)*s. The {-1,0,1} ternary part
  is exact in fp8 (and the e4m3 x upcasts exactly through the DR e6m3 path);
  per-out-channel scale s and BN fold into (a, c).
- h1 is stored e4m3 with the image-pair/channel-half pixels interleaved
  (offset r*64 + c*4 + i*2 + e) so the DoubleRow moving operand reads its fp8
  pair in one 16-bit access — without this DR matmuls run at half speed.
- A short dummy-matmul block covers the first xq chunk's DMA lead-in and
  starts the HAM clock-gate warmup; mid-kernel PE gaps are all <1.5us so no
  keep-warm matmuls are needed (HAM re-throttles only after ~3.4us idle).
"""
import numpy as np
import ml_dtypes

import concourse.bass as bass
import concourse.mybir as mybir
from concourse import bacc
from concourse.tile import TileContext
from concourse.bass_utils import run_bass_kernel_spmd
from concourse.masks import make_identity


BN_EPS = 1e-5
Q_EPS = 1e-8

# Problem shape (hardcoded per contract)
B, CIN, H, W = 64, 1024, 14, 14
WIDTH, COUT = 256, 1024
N_CORES = 8
BC = B // N_CORES          # images per core = 8
PIX = H * W                # 196
P = 128
CIN_T = CIN // P           # 8
W_T = WIDTH // P           # 2
COUT_T = COUT // P         # 8
IPG = 2                    # images per matmul group (pixel-interleaved)
BP = BC // IPG             # 4 image-pair groups per core
NN = IPG * PIX             # 392 columns per matmul
HP, WP = H + 2, W + 2      # 16x16 padded image for the 3x3 conv
KP1 = CIN_T // 2           # 4 DoubleRow k-pairs for conv1
XB = CIN_T * NN            # bf16 x columns per bp (3136)
XQ = KP1 * NN * 2          # fp8 xq columns per bp (3136)
NDUMMY = 85                # 64-col pre-warm matmuls (cover the DMA lead-in)

_F32 = mybir.dt.float32
_BF16 = mybir.dt.bfloat16
_FP8 = mybir.dt.float8e4
_AF = mybir.ActivationFunctionType
_ALU = mybir.AluOpType
_DR = mybir.MatmulPerfMode.DoubleRow


def build(act_func=None):
    """Build the per-core Bass program (SPMD: same program on all 8 cores)."""
    if act_func is None:
        act_func = _AF.Silu
    nc = bacc.Bacc()

    # bp-major bf16 x (residual only): xd[p, bp*XB + k*NN + n], n = (r*W+c)*2+i
    xd = nc.declare_dram_parameter("x", [P, BP * XB], _BF16, isOutput=False)
    # bp-major e4m3 x for conv1's DoubleRow k-pairs:
    # xq[p, bp*XQ + kp*2*NN + n*2 + e], ch = (2*kp+e)*128+p
    xqd = nc.declare_dram_parameter("xq", [P, BP * XQ], _FP8, isOutput=False)
    w1d = nc.declare_dram_parameter("w1", [P, CIN_T * WIDTH], _FP8, isOutput=False)
    w2d = nc.declare_dram_parameter("w2", [P, W_T * 9 * WIDTH], _FP8, isOutput=False)
    w3d = nc.declare_dram_parameter("w3", [P, W_T * COUT], _BF16, isOutput=False)
    ccd = nc.declare_dram_parameter("cc", [P, 8 + COUT_T], _F32, isOutput=False)
    # output stays in the partition-folded layout [p, j*BC*PIX + n]; host unfolds
    outd = nc.declare_dram_parameter("out", [P, COUT_T * BC * PIX], _BF16, isOutput=True)

    with TileContext(nc) as tc:
        with tc.tile_pool(name="weights", bufs=1) as wpool, \
             tc.tile_pool(name="acts", bufs=1) as apool, \
             tc.tile_pool(name="outs", bufs=4) as opool, \
             tc.tile_pool(name="stage", bufs=3) as stpool, \
             tc.tile_pool(name="psum", bufs=2, space="PSUM") as pspool:

            # ---- one sync DMA queue, issue order = priority order (an issue
            # costs ~0.7us of engine time). conv1 needs only w1+xq, so l1 of
            # all bps runs off the front of the stream; bf16 x (residual) and
            # later weights ride behind. Out DMAs reuse this queue from ~20us
            # (it is idle by then). cc rides gpsimd in parallel. ----
            w1t = wpool.tile([P, CIN_T * WIDTH], _FP8, name="w1t")
            xt = apool.tile([P, BP * XB], _BF16, name="xt")
            xqt = apool.tile([P, BP * XQ], _FP8, name="xqt")

            def dma_xq(bp):
                a, b = bp * XQ, (bp + 1) * XQ
                nc.sync.dma_start(out=xqt[:, a:b], in_=xqd[:, a:b])

            def dma_x(bp, k0, k1):
                a, b = bp * XB + k0 * NN, bp * XB + k1 * NN
                nc.sync.dma_start(out=xt[:, a:b], in_=xd[:, a:b])

            # xq0 first (smallest critical chunk -> earliest completion sem),
            # then w1 in two halves so l1's kp0/kp1 matmuls can fire while
            # kp2/kp3 weights are still in flight
            dma_xq(0)
            nc.sync.dma_start(out=w1t[:, 0:4 * WIDTH], in_=w1d[:, 0:4 * WIDTH])
            nc.sync.dma_start(out=w1t[:, 4 * WIDTH:], in_=w1d[:, 4 * WIDTH:])
            dma_xq(1)
            w2t = wpool.tile([P, W_T * 9 * WIDTH], _FP8, name="w2t")
            nc.sync.dma_start(out=w2t[:, :], in_=w2d[:, :])
            dma_xq(2)
            dma_xq(3)
            w3t = wpool.tile([P, W_T * COUT], _BF16, name="w3t")
            nc.sync.dma_start(out=w3t[:, :], in_=w3d[:, :])
            dma_x(0, 0, 8)
            dma_x(1, 0, 8)
            dma_x(2, 0, 8)
            dma_x(3, 0, 8)
            cct = wpool.tile([P, 8 + COUT_T], _F32, name="cct")
            nc.gpsimd.dma_start(out=cct[:, :], in_=ccd[:, :])

            def xs(bp, t):          # x slice [128, NN] for (bpair, channel tile)
                return xt[:, bp * XB + t * NN: bp * XB + (t + 1) * NN]

            # ---- PE clock pre-warm: HAM needs ~3.4us of sustained PE activity
            # to lift the 1.2->2.4GHz clock gate; dummy matmuls bridge the
            # first xq chunk's DMA lead-in ----
            wsrc = apool.tile([P, 128], _BF16, name="wsrc")
            nc.vector.memset(wsrc[:, :], 0.0)
            identt = wpool.tile([P, P], _BF16, name="identt")
            make_identity(nc, identt[:, :])
            wps = pspool.tile([P, 1024], _F32, name="wps", tag="ps")
            for _ in range(NDUMMY):
                nc.tensor.matmul(wps[0:64, 0:64], wsrc[:, 0:64],
                                 wsrc[:, 0:64], start=True, stop=True)

            # ---- padded h1 buffers, fp8. Layout per bp group:
            # offset = r*64 + c*4 + i*2 + e  (e = channel half INNERMOST so
            # the DoubleRow moving operand reads its fp8 pair in one 16-bit
            # access) ----
            h1p = []
            for bp in range(BP):
                t = apool.tile([P, HP * WP * IPG * W_T], _FP8, name=f"h1p{bp}")
                nc.vector.memset(t[:, :], 0.0)
                h1p.append(t)
            h2 = [apool.tile([P, W_T * NN], _BF16, name=f"h2_{bp}")
                  for bp in range(BP)]

            w1v = w1t.rearrange("p (k m) -> p k m", k=CIN_T)
            xqv = xqt.rearrange("p (bp kp n e) -> p bp kp e n", bp=BP, kp=KP1,
                                e=2)
            w2v = w2t.rearrange("p (j t e m) -> p j t e m", j=W_T, t=9, e=W_T)

            # ---- stage A: layer 1 for one (bp, j) half. 1x1 conv 1024->256
            # as four DoubleRow e4m3 k-pairs into the half's OWN single-bank
            # psum tile (a shared 2-bank tile would let the framework
            # serialize j1's matmuls behind j0's ACT read), then one ACT
            # silu(a1*psum + c1) straight from PSUM into the padded fp8 h1 ----
            def l1_half(bp, j):
                ps1 = pspool.tile([P, 512], _F32, name=f"ps1_{bp}{j}",
                                  tag="ps1")
                for kp in range(KP1):
                    nc.tensor.matmul(
                        ps1[:, 0:NN],
                        w1v[:, 2 * kp:2 * kp + 2, j * P:(j + 1) * P],
                        xqv[:, bp, kp, :, :],
                        start=(kp == 0), stop=(kp == KP1 - 1),
                        perf_mode=_DR)
                src = ps1[:, 0:NN].rearrange("p (r c i) -> p r c i", r=H, c=W)
                dst = h1p[bp].rearrange(
                    "p (r c i e) -> p r c i e", r=HP, c=WP,
                    i=IPG)[:, 1:1 + H, 1:1 + W, :, j]
                nc.scalar.activation(dst, src, act_func,
                                     bias=cct[:, 2 + j:3 + j],
                                     scale=cct[:, 0 + j:1 + j])

            def l1_stage(bp):
                l1_half(bp, 0)
                l1_half(bp, 1)

            # ---- stages B/C: layer 2, 3x3 conv 256->256 via 9 shifted-tap
            # DoubleRow fp8 matmuls per output j-tile (B = j0, C = j1), each
            # j into its own single-bank psum tile. The per-j ACT
            # silu(a2*psum + c2) psum->h2 follows its own taps, so it
            # overlaps the next block's matmuls ----
            def l2_stage(bp, j):
                ps2 = pspool.tile([P, 512], _F32, name=f"ps2_{bp}{j}",
                                  tag="ps2")
                for tap in range(9):
                    dy, dx = divmod(tap, 3)
                    rhs = h1p[bp].rearrange(
                        "p (r ci e) -> p e r ci", r=HP, e=W_T
                    )[:, :, dy:dy + H, IPG * dx:IPG * dx + IPG * W]
                    nc.tensor.matmul(
                        ps2[:, 0:NN],
                        w2v[:, j, tap], rhs,
                        start=(tap == 0), stop=(tap == 8), perf_mode=_DR)
                nc.scalar.activation(h2[bp][:, j * NN:(j + 1) * NN],
                                     ps2[:, 0:NN],
                                     act_func, bias=cct[:, 6 + j:7 + j],
                                     scale=cct[:, 4 + j:5 + j])

            # ---- stages D/E: layer 3, 1x1 conv 256->1024 bf16 (a3 folded into
            # weights). One "unit" = a j-pair: 4 conv matmuls + epilogue.
            # The residual x already carries c3 (folded on the host), so the
            # epilogue per j-pair is either ONE pair-batched DVE add
            # psum->stage + a quad pure-silu ACT (mode 'stage' — no PE work),
            # or an identity matmul on the PE with a pair ACT straight from
            # PSUM (mode 'pe' — used in the bp3 tail where the PE would idle).
            # dma='pair' flushes half-size DMAs to shorten the tail. ----
            def l3_units(bp, modes='ssss', dma='quad', dma_eng=None):
                state = {}

                def mk(j0, mode):
                    def emit():
                        if mode == 'stage' and j0 % 4 == 0:
                            state['st'] = stpool.tile([P, 4 * NN], _F32,
                                                      name="st", tag="st")
                        if j0 % 4 == 0:
                            state['ot'] = opool.tile([P, 4 * NN], _BF16,
                                                     name="ot", tag="ot")
                        ot = state['ot']
                        ps3 = pspool.tile([P, 1024], _F32, name="ps3", tag="ps")
                        for dj in range(2):
                            j = j0 + dj
                            sl = ps3[:, dj * 512: dj * 512 + NN]
                            for k in range(W_T):
                                nc.tensor.matmul(
                                    sl,
                                    w3t[:, k * COUT + j * P:
                                        k * COUT + (j + 1) * P],
                                    h2[bp][:, k * NN:(k + 1) * NN],
                                    start=(k == 0),
                                    stop=(mode == 'stage' and k == W_T - 1))
                            if mode == 'pe':
                                nc.tensor.matmul(sl, identt[:, :], xs(bp, j),
                                                 start=False, stop=True)
                        pspair = ps3.rearrange("p (g n) -> p g n", g=2)[
                            :, :, 0:NN]
                        otpair = ot.rearrange("p (g n) -> p g n", g=4)[
                            :, (j0 % 4):(j0 % 4) + 2, :]
                        if mode == 'stage':
                            xpair = xt.rearrange(
                                "p (bp t n) -> p bp t n", bp=BP, t=CIN_T)[
                                :, bp, j0:j0 + 2, :]
                            stpair = state['st'][
                                :, (j0 % 4) * NN:(j0 % 4 + 2) * NN
                            ].rearrange("p (g n) -> p g n", g=2)
                            nc.vector.tensor_add(out=stpair, in0=pspair,
                                                 in1=xpair)
                            if dma == 'pair':
                                nc.scalar.activation(otpair, stpair, act_func)
                            elif j0 % 4 == 2:
                                nc.scalar.activation(ot[:, :],
                                                     state['st'][:, :],
                                                     act_func)
                        else:
                            nc.scalar.activation(otpair, pspair, act_func)
                        if dma == 'pair' or j0 % 4 == 2:
                            jlo = j0 if dma == 'pair' else j0 - 2
                            nj = 2 if dma == 'pair' else 4
                            dmadst = outd.rearrange(
                                "p (j n) -> p j n", j=COUT_T)[
                                :, jlo:jlo + nj, bp * NN:(bp + 1) * NN]
                            src = ot.rearrange("p (g n) -> p g n", g=4)[
                                :, (jlo % 4):(jlo % 4) + nj, :]
                            (dma_eng or nc.sync).dma_start(out=dmadst, in_=src)
                    return emit
                return [mk(j0, {'s': 'stage', 'p': 'pe'}[m])
                        for j0, m in zip((0, 2, 4, 6), modes)]

            # ---- software-pipelined emission across the 4 bps ----
            units = {bp: l3_units(bp, modes='ssss') for bp in range(3)}
            units[3] = l3_units(3, modes='ssss', dma='pair')

            def D(bp):
                units[bp][0]()
                units[bp][1]()

            def E(bp):
                units[bp][2]()
                units[bp][3]()

            l1_stage(0)
            l1_stage(1)
            l2_stage(0, 0)
            l2_stage(0, 1)
            l2_stage(1, 0)
            l2_stage(1, 1)
            l1_stage(2)
            D(0)
            l2_stage(2, 0)
            E(0)
            l2_stage(2, 1)
            l1_half(3, 0)
            D(1)
            l1_half(3, 1)
            E(1)
            l2_stage(3, 0)
            l2_stage(3, 1)
            D(2)
            E(2)
            D(3)
            E(3)

    nc.finalize()
    return nc


def _prep_host(x, w1, b1, g1, be1, m1, v1,
               w2, b2, g2, be2, m2, v2,
               w3, b3, g3, be3, m3, v3):
    """Quantize weights, fold BN, and lay out device arrays."""
    def quant(w):
        w = np.asarray(w, np.float32)
        s = np.median(np.abs(w).reshape(w.shape[0], -1), axis=1)
        s = np.maximum(s, np.float32(Q_EPS)).astype(np.float32)
        t = np.clip(np.round(w / s[:, None, None, None]), -1.0, 1.0).astype(np.float32)
        return t, s

    def fold(s, b, g, be, m, v):
        sc = np.asarray(g, np.float64) / np.sqrt(np.asarray(v, np.float64) + BN_EPS)
        a = (np.asarray(s, np.float64) * sc).astype(np.float32)
        c = (np.asarray(b, np.float64) * sc + np.asarray(be, np.float64)
             - np.asarray(m, np.float64) * sc).astype(np.float32)
        return a, c

    t1, s1 = quant(w1)
    t2, s2 = quant(w2)
    t3, s3 = quant(w3)
    a1, c1 = fold(s1, b1, g1, be1, m1, v1)
    a2, c2 = fold(s2, b2, g2, be2, m2, v2)
    a3, c3 = fold(s3, b3, g3, be3, m3, v3)

    # Fold c3 into the residual input: ship x' = x + c3 (broadcast per
    # channel; CIN == COUT so the same tensor serves conv1 and the residual).
    # conv1 then sees a per-channel constant shift, corrected exactly in c1:
    # conv1(x + c3) = conv1(x) + t1 @ c3  =>  c1 -= a1 * (t1 @ c3)
    t1c3 = t1[:, :, 0, 0].astype(np.float64) @ c3.astype(np.float64)
    c1 = (c1.astype(np.float64) - a1.astype(np.float64) * t1c3).astype(np.float32)

    bf = ml_dtypes.bfloat16
    fp8 = ml_dtypes.float8_e4m3

    def part_fold(m2d):
        # [K, M] -> [128, (K//128)*M]: row k*128+p lands at [p, k*M+m]
        kk, mm = m2d.shape
        return np.ascontiguousarray(
            m2d.reshape(kk // P, P, mm).transpose(1, 0, 2).reshape(P, -1))

    w1_dev = part_fold(t1[:, :, 0, 0].T).astype(fp8)
    # w2_dev[p, j, tap, e, m] = t2[j*128+m, e*128+p, dy, dx]
    w2_dev = np.ascontiguousarray(
        t2.reshape(W_T, P, W_T, P, 3, 3)           # j, m, e, p, dy, dx
        .transpose(3, 0, 4, 5, 2, 1)               # p, j, dy, dx, e, m
        .reshape(P, W_T * 9 * W_T * P)).astype(fp8)
    w3_dev = part_fold((t3[:, :, 0, 0] * a3[:, None]).T).astype(bf)

    cc = np.zeros((P, 8 + COUT_T), np.float32)
    cc[:, 0:2] = a1.reshape(W_T, P).T
    cc[:, 2:4] = c1.reshape(W_T, P).T
    cc[:, 4:6] = a2.reshape(W_T, P).T
    cc[:, 6:8] = c2.reshape(W_T, P).T
    cc[:, 8:] = c3.reshape(COUT_T, P).T

    const = {"w1": w1_dev, "w2": w2_dev, "w3": w3_dev,
             "cc": np.ascontiguousarray(cc)}

    x = np.asarray(x, np.float32) + c3[None, :, None, None]
    in_maps = []
    for c in range(N_CORES):
        # xd[p, bp*XB + k*NN + (r*W+cw)*2 + i] = x[c*BC+bp*2+i, k*128+p, r, cw]
        xb = x[c * BC:(c + 1) * BC].reshape(BP, IPG, CIN_T, P, H, W)
        xc = np.ascontiguousarray(
            xb.transpose(3, 0, 2, 4, 5, 1).reshape(P, BP * XB))
        # xq[p, bp*XQ + kp*2*NN + ((r*W+cw)*2+i)*2 + e] = x'[ch=(2*kp+e)*128+p]
        xq = xb.reshape(BP, IPG, KP1, 2, P, H, W)
        xq = np.ascontiguousarray(
            xq.transpose(4, 0, 2, 5, 6, 1, 3).reshape(P, BP * XQ))
        in_maps.append({"x": xc.astype(bf), "xq": xq.astype(fp8), **const})
    return in_maps


def _run(inputs, trace=False, act_func=None, **spmd_kwargs):
    nc = build(act_func)
    in_maps = _prep_host(**inputs)
    res = run_bass_kernel_spmd(nc, in_maps, list(range(N_CORES)),
                               trace=trace, **spmd_kwargs)
    outs = []
    for c in range(N_CORES):
        of = res.results[c]["out"].astype(np.float32)    # folded [P, COUT_T*BC*PIX]
        oc = of.reshape(P, COUT_T, BP, H, W, IPG)
        oc = oc.transpose(2, 5, 1, 0, 3, 4).reshape(BC, COUT, H, W)
        outs.append(oc)
    full = np.concatenate(outs, axis=0).astype(np.float32)
    return full, res


def kernel(**inputs):
    out, _ = _run(inputs)
    return out


# revision 35
# speedup vs baseline: 1.0619x; 1.0072x over previous
"""Trainium2 Bass kernel for nn_BottleneckBit (ResNet bottleneck with ternary-
quantized convs + BN + SiLU + residual).

Strategy:
- Data-parallel over batch: 64 images -> 8 cores x 8 images.
- All convs lowered to TensorEngine matmuls with channels on partitions:
    conv1 (1x1, 1024->256):  four DoubleRow e4m3 k-pairs — the full 1024-ch
                             contraction runs at fp8 rate (x quantization
                             error budgeted via simulation: rel 1.80e-2 of
                             the 2e-2 gate, validated bit-exact against HW
                             on the mixed variant)
    conv2 (3x3, 256->256):   DoubleRow fp8: both 128-channel halves contract in
                             one pass. 9 shifted-tap DR matmuls per output
                             tile, reading a zero-padded 16x16-per-image fp8
                             buffer with image-pair pixels interleaved
                             (n = r*28 + c*2 + i) so the shifted window is a
                             4D AP [p, half, r, colpair].
    conv3 (1x1, 256->1024):  2 K-tiles bf16, a3 folded into the weights and c3
                             folded into the residual x' = x + c3 on the host
                             (conv1 corrected exactly via c1' = c1 - a1*(t1@c3),
                             CIN == COUT).
- BN affines ride the ACT engine: silu(a*psum + c) is one scalar.activation
  with per-partition scale/bias APs reading PSUM directly — no DVE affine in
  the l1/l2 epilogue chains, and per-j ACTs are emitted as soon as their
  PSUM half is stopped so the chain after the last matmul is one 392-col ACT.
- Batch-pair-major dataflow: x/xq are bp-major in DRAM. Because conv1 needs
  only the small fp8 xq (0.2MB/bp), l1 of all 4 bps runs off the front of the
  input stream; the bf16 x (residual-only) streams behind it. Stages
      A=l1, B=l2 taps j0 (+ACT j0), C=l2 taps j1 (+ACT j1),
      D=l3 units j0-3, E=l3 units j4-7
  are software-pipelined A0 A1 B0 C0 B1 C1 A2 D0 E0 B2 C2 A3 D1 E1 B3 C3
  D2 E2 D3 E3 — every producer->consumer epilogue hides under >=1
  intervening stage of PE work and the out stream starts ~1/3 in.
- Single sync-queue DMA with issue-order = priority order (each dma_start
  costs ~0.7us of issue time on its engine, so queue assignment matters):
  w1, xq0, w2, xq1, xq2, xq3, w3, then the residual x halves; out DMAs are
  also on sync (it is idle mid-kernel; they must not ride the busy ACT
  queue). Only cc rides gpsimd.
- l3 epilogue: residual + silu via one pair-batched DVE tensor_add
  psum->stage (the DVE is the psum tile's last reader, so psum recycling
  never waits on the busier ACT queue) + a quad pure-silu ACT for bp0..2;
  the bp3 tail keeps the same 'stage' mode but with per-unit pair ACTs and
  pair DMAs so the final outputs stream out as soon as each j-pair is done.
- Ternary weight trick: wq = clip(round(w/s),-1,1)*s. The {-1,0,1} ternary part
  is exact in fp8 (and the e4m3 x upcasts exactly through the DR e6m3 path);
  per-out-channel scale s and BN fold into (a, c).
- h1 is stored e4m3 with the image-pair/channel-half pixels interleaved
  (offset r*64 + c*4 + i*2 + e) so the DoubleRow moving operand reads its fp8
  pair in one 16-bit access — without this DR matmuls run at half speed.
- A short dummy-matmul block covers the first xq chunk's DMA lead-in and
  starts the HAM clock-gate warmup; mid-kernel PE gaps are all <1.5us so no
  keep-warm matmuls are needed (HAM re-throttles only after ~3.4us idle).
"""
import numpy as np
import ml_dtypes

import concourse.bass as bass
import concourse.mybir as mybir
from concourse import bacc
from concourse.tile import TileContext
from concourse.bass_utils import run_bass_kernel_spmd
from concourse.masks import make_identity


BN_EPS = 1e-5
Q_EPS = 1e-8

# Problem shape (hardcoded per contract)
B, CIN, H, W = 64, 1024, 14, 14
WIDTH, COUT = 256, 1024
N_CORES = 8
BC = B // N_CORES          # images per core = 8
PIX = H * W                # 196
P = 128
CIN_T = CIN // P           # 8
W_T = WIDTH // P           # 2
COUT_T = COUT // P         # 8
IPG = 2                    # images per matmul group (pixel-interleaved)
BP = BC // IPG             # 4 image-pair groups per core
NN = IPG * PIX             # 392 columns per matmul
HP, WP = H + 2, W + 2      # 16x16 padded image for the 3x3 conv
KP1 = CIN_T // 2           # 4 DoubleRow k-pairs for conv1
XB = CIN_T * NN            # bf16 x columns per bp (3136)
XQ = KP1 * NN * 2          # fp8 xq columns per bp (3136)
NDUMMY = 85                # 64-col pre-warm matmuls (cover the DMA lead-in)

_F32 = mybir.dt.float32
_BF16 = mybir.dt.bfloat16
_FP8 = mybir.dt.float8e4
_AF = mybir.ActivationFunctionType
_ALU = mybir.AluOpType
_DR = mybir.MatmulPerfMode.DoubleRow


def build(act_func=None):
    """Build the per-core Bass program (SPMD: same program on all 8 cores)."""
    if act_func is None:
        act_func = _AF.Silu
    nc = bacc.Bacc()

    # bp-major bf16 x (residual only): xd[p, bp*XB + k*NN + n], n = (r*W+c)*2+i
    xd = nc.declare_dram_parameter("x", [P, BP * XB], _BF16, isOutput=False)
    # bp-major e4m3 x for conv1's DoubleRow k-pairs:
    # xq[p, bp*XQ + kp*2*NN + n*2 + e], ch = (2*kp+e)*128+p
    xqd = nc.declare_dram_parameter("xq", [P, BP * XQ], _FP8, isOutput=False)
    w1d = nc.declare_dram_parameter("w1", [P, CIN_T * WIDTH], _FP8, isOutput=False)
    w2d = nc.declare_dram_parameter("w2", [P, W_T * 9 * WIDTH], _FP8, isOutput=False)
    w3d = nc.declare_dram_parameter("w3", [P, W_T * COUT], _BF16, isOutput=False)
    ccd = nc.declare_dram_parameter("cc", [P, 8 + COUT_T], _F32, isOutput=False)
    # output stays in the partition-folded layout [p, j*BC*PIX + n]; host unfolds
    outd = nc.declare_dram_parameter("out", [P, COUT_T * BC * PIX], _BF16, isOutput=True)

    with TileContext(nc) as tc:
        with tc.tile_pool(name="weights", bufs=1) as wpool, \
             tc.tile_pool(name="acts", bufs=1) as apool, \
             tc.tile_pool(name="outs", bufs=4) as opool, \
             tc.tile_pool(name="stage", bufs=3) as stpool, \
             tc.tile_pool(name="psum", bufs=2, space="PSUM") as pspool:

            # ---- one sync DMA queue, issue order = priority order (an issue
            # costs ~0.7us of engine time). conv1 needs only w1+xq, so l1 of
            # all bps runs off the front of the stream; bf16 x (residual) and
            # later weights ride behind. Out DMAs reuse this queue from ~20us
            # (it is idle by then). cc rides gpsimd in parallel. ----
            w1t = wpool.tile([P, CIN_T * WIDTH], _FP8, name="w1t")
            xt = apool.tile([P, BP * XB], _BF16, name="xt")
            xqt = apool.tile([P, BP * XQ], _FP8, name="xqt")

            def dma_xq(bp):
                a, b = bp * XQ, (bp + 1) * XQ
                nc.sync.dma_start(out=xqt[:, a:b], in_=xqd[:, a:b])

            def dma_x(bp, k0, k1):
                a, b = bp * XB + k0 * NN, bp * XB + k1 * NN
                nc.sync.dma_start(out=xt[:, a:b], in_=xd[:, a:b])

            # xq0 first (smallest critical chunk -> earliest completion sem).
            # Each dma_start costs ~0.6-1.3us of issue time on the sync queue,
            # so later-needed chunks are merged into as few DMAs as possible.
            dma_xq(0)
            nc.sync.dma_start(out=w1t[:, :], in_=w1d[:, :])
            dma_xq(1)
            w2t = wpool.tile([P, W_T * 9 * WIDTH], _FP8, name="w2t")
            nc.sync.dma_start(out=w2t[:, :], in_=w2d[:, :])
            nc.sync.dma_start(out=xqt[:, 2 * XQ:], in_=xqd[:, 2 * XQ:])
            w3t = wpool.tile([P, W_T * COUT], _BF16, name="w3t")
            nc.sync.dma_start(out=w3t[:, :], in_=w3d[:, :])
            dma_x(0, 0, 8)
            dma_x(1, 0, 8)
            dma_x(2, 0, 8)
            dma_x(3, 0, 8)
            cct = wpool.tile([P, 8 + COUT_T], _F32, name="cct")
            nc.gpsimd.dma_start(out=cct[:, :], in_=ccd[:, :])

            def xs(bp, t):          # x slice [128, NN] for (bpair, channel tile)
                return xt[:, bp * XB + t * NN: bp * XB + (t + 1) * NN]

            # ---- PE clock pre-warm: HAM needs ~3.4us of sustained PE activity
            # to lift the 1.2->2.4GHz clock gate; dummy matmuls bridge the
            # first xq chunk's DMA lead-in ----
            wsrc = apool.tile([P, 128], _BF16, name="wsrc")
            nc.vector.memset(wsrc[:, :], 0.0)
            identt = wpool.tile([P, P], _BF16, name="identt")
            make_identity(nc, identt[:, :])
            wps = pspool.tile([P, 1024], _F32, name="wps", tag="ps")
            for _ in range(NDUMMY):
                nc.tensor.matmul(wps[0:64, 0:64], wsrc[:, 0:64],
                                 wsrc[:, 0:64], start=True, stop=True)

            # ---- padded h1 buffers, fp8. Layout per bp group:
            # offset = r*64 + c*4 + i*2 + e  (e = channel half INNERMOST so
            # the DoubleRow moving operand reads its fp8 pair in one 16-bit
            # access) ----
            h1p = []
            for bp in range(BP):
                t = apool.tile([P, HP * WP * IPG * W_T], _FP8, name=f"h1p{bp}")
                nc.vector.memset(t[:, :], 0.0)
                h1p.append(t)
            h2 = [apool.tile([P, W_T * NN], _BF16, name=f"h2_{bp}")
                  for bp in range(BP)]

            w1v = w1t.rearrange("p (k m) -> p k m", k=CIN_T)
            xqv = xqt.rearrange("p (bp kp n e) -> p bp kp e n", bp=BP, kp=KP1,
                                e=2)
            w2v = w2t.rearrange("p (j t e m) -> p j t e m", j=W_T, t=9, e=W_T)

            # ---- stage A: layer 1 for one (bp, j) half. 1x1 conv 1024->256
            # as four DoubleRow e4m3 k-pairs into the half's OWN single-bank
            # psum tile (a shared 2-bank tile would let the framework
            # serialize j1's matmuls behind j0's ACT read), then one ACT
            # silu(a1*psum + c1) straight from PSUM into the padded fp8 h1 ----
            def l1_half(bp, j):
                ps1 = pspool.tile([P, 512], _F32, name=f"ps1_{bp}{j}",
                                  tag="ps1")
                for kp in range(KP1):
                    nc.tensor.matmul(
                        ps1[:, 0:NN],
                        w1v[:, 2 * kp:2 * kp + 2, j * P:(j + 1) * P],
                        xqv[:, bp, kp, :, :],
                        start=(kp == 0), stop=(kp == KP1 - 1),
                        perf_mode=_DR)
                src = ps1[:, 0:NN].rearrange("p (r c i) -> p r c i", r=H, c=W)
                dst = h1p[bp].rearrange(
                    "p (r c i e) -> p r c i e", r=HP, c=WP,
                    i=IPG)[:, 1:1 + H, 1:1 + W, :, j]
                nc.scalar.activation(dst, src, act_func,
                                     bias=cct[:, 2 + j:3 + j],
                                     scale=cct[:, 0 + j:1 + j])

            def l1_stage(bp):
                l1_half(bp, 0)
                l1_half(bp, 1)

            # ---- stages B/C: layer 2, 3x3 conv 256->256 via 9 shifted-tap
            # DoubleRow fp8 matmuls per output j-tile (B = j0, C = j1), each
            # j into its own single-bank psum tile. The per-j ACT
            # silu(a2*psum + c2) psum->h2 follows its own taps, so it
            # overlaps the next block's matmuls ----
            def l2_stage(bp, j):
                ps2 = pspool.tile([P, 512], _F32, name=f"ps2_{bp}{j}",
                                  tag="ps2")
                for tap in range(9):
                    dy, dx = divmod(tap, 3)
                    rhs = h1p[bp].rearrange(
                        "p (r ci e) -> p e r ci", r=HP, e=W_T
                    )[:, :, dy:dy + H, IPG * dx:IPG * dx + IPG * W]
                    nc.tensor.matmul(
                        ps2[:, 0:NN],
                        w2v[:, j, tap], rhs,
                        start=(tap == 0), stop=(tap == 8), perf_mode=_DR)
                nc.scalar.activation(h2[bp][:, j * NN:(j + 1) * NN],
                                     ps2[:, 0:NN],
                                     act_func, bias=cct[:, 6 + j:7 + j],
                                     scale=cct[:, 4 + j:5 + j])

            # ---- stages D/E: layer 3, 1x1 conv 256->1024 bf16 (a3 folded into
            # weights). One "unit" = a j-pair: 4 conv matmuls + epilogue.
            # The residual x already carries c3 (folded on the host), so the
            # epilogue per j-pair is either ONE pair-batched DVE add
            # psum->stage + a quad pure-silu ACT (mode 'stage' — no PE work),
            # or an identity matmul on the PE with a pair ACT straight from
            # PSUM (mode 'pe' — used in the bp3 tail where the PE would idle).
            # dma='pair' flushes half-size DMAs to shorten the tail. ----
            def l3_units(bp, modes='ssss', dma='quad', dma_eng=None):
                state = {}

                def mk(j0, mode):
                    def emit():
                        if mode == 'stage' and j0 % 4 == 0:
                            state['st'] = stpool.tile([P, 4 * NN], _F32,
                                                      name="st", tag="st")
                        if j0 % 4 == 0:
                            state['ot'] = opool.tile([P, 4 * NN], _BF16,
                                                     name="ot", tag="ot")
                        ot = state['ot']
                        ps3 = pspool.tile([P, 1024], _F32, name="ps3", tag="ps")
                        for dj in range(2):
                            j = j0 + dj
                            sl = ps3[:, dj * 512: dj * 512 + NN]
                            for k in range(W_T):
                                nc.tensor.matmul(
                                    sl,
                                    w3t[:, k * COUT + j * P:
                                        k * COUT + (j + 1) * P],
                                    h2[bp][:, k * NN:(k + 1) * NN],
                                    start=(k == 0),
                                    stop=(mode == 'stage' and k == W_T - 1))
                            if mode == 'pe':
                                nc.tensor.matmul(sl, identt[:, :], xs(bp, j),
                                                 start=False, stop=True)
                        pspair = ps3.rearrange("p (g n) -> p g n", g=2)[
                            :, :, 0:NN]
                        otpair = ot.rearrange("p (g n) -> p g n", g=4)[
                            :, (j0 % 4):(j0 % 4) + 2, :]
                        if mode == 'stage':
                            xpair = xt.rearrange(
                                "p (bp t n) -> p bp t n", bp=BP, t=CIN_T)[
                                :, bp, j0:j0 + 2, :]
                            stpair = state['st'][
                                :, (j0 % 4) * NN:(j0 % 4 + 2) * NN
                            ].rearrange("p (g n) -> p g n", g=2)
                            nc.vector.tensor_add(out=stpair, in0=pspair,
                                                 in1=xpair)
                            if dma == 'pair':
                                nc.scalar.activation(otpair, stpair, act_func)
                            elif j0 % 4 == 2:
                                nc.scalar.activation(ot[:, :],
                                                     state['st'][:, :],
                                                     act_func)
                        else:
                            nc.scalar.activation(otpair, pspair, act_func)
                        if dma == 'pair' or j0 % 4 == 2:
                            jlo = j0 if dma == 'pair' else j0 - 2
                            nj = 2 if dma == 'pair' else 4
                            dmadst = outd.rearrange(
                                "p (j n) -> p j n", j=COUT_T)[
                                :, jlo:jlo + nj, bp * NN:(bp + 1) * NN]
                            src = ot.rearrange("p (g n) -> p g n", g=4)[
                                :, (jlo % 4):(jlo % 4) + nj, :]
                            (dma_eng or nc.sync).dma_start(out=dmadst, in_=src)
                    return emit
                return [mk(j0, {'s': 'stage', 'p': 'pe'}[m])
                        for j0, m in zip((0, 2, 4, 6), modes)]

            # ---- software-pipelined emission across the 4 bps ----
            units = {bp: l3_units(bp, modes='ssss') for bp in range(3)}
            units[3] = l3_units(3, modes='ssss', dma='pair')

            def D(bp):
                units[bp][0]()
                units[bp][1]()

            def E(bp):
                units[bp][2]()
                units[bp][3]()

            l1_stage(0)
            l1_stage(1)
            l2_stage(0, 0)
            l2_stage(0, 1)
            l2_stage(1, 0)
            l2_stage(1, 1)
            l1_stage(2)
            D(0)
            l2_stage(2, 0)
            E(0)
            l2_stage(2, 1)
            l1_half(3, 0)
            D(1)
            l1_half(3, 1)
            E(1)
            l2_stage(3, 0)
            l2_stage(3, 1)
            D(2)
            E(2)
            D(3)
            E(3)

    nc.finalize()
    return nc


def _prep_host(x, w1, b1, g1, be1, m1, v1,
               w2, b2, g2, be2, m2, v2,
               w3, b3, g3, be3, m3, v3):
    """Quantize weights, fold BN, and lay out device arrays."""
    def quant(w):
        w = np.asarray(w, np.float32)
        s = np.median(np.abs(w).reshape(w.shape[0], -1), axis=1)
        s = np.maximum(s, np.float32(Q_EPS)).astype(np.float32)
        t = np.clip(np.round(w / s[:, None, None, None]), -1.0, 1.0).astype(np.float32)
        return t, s

    def fold(s, b, g, be, m, v):
        sc = np.asarray(g, np.float64) / np.sqrt(np.asarray(v, np.float64) + BN_EPS)
        a = (np.asarray(s, np.float64) * sc).astype(np.float32)
        c = (np.asarray(b, np.float64) * sc + np.asarray(be, np.float64)
             - np.asarray(m, np.float64) * sc).astype(np.float32)
        return a, c

    t1, s1 = quant(w1)
    t2, s2 = quant(w2)
    t3, s3 = quant(w3)
    a1, c1 = fold(s1, b1, g1, be1, m1, v1)
    a2, c2 = fold(s2, b2, g2, be2, m2, v2)
    a3, c3 = fold(s3, b3, g3, be3, m3, v3)

    # Fold c3 into the residual input: ship x' = x + c3 (broadcast per
    # channel; CIN == COUT so the same tensor serves conv1 and the residual).
    # conv1 then sees a per-channel constant shift, corrected exactly in c1:
    # conv1(x + c3) = conv1(x) + t1 @ c3  =>  c1 -= a1 * (t1 @ c3)
    t1c3 = t1[:, :, 0, 0].astype(np.float64) @ c3.astype(np.float64)
    c1 = (c1.astype(np.float64) - a1.astype(np.float64) * t1c3).astype(np.float32)

    bf = ml_dtypes.bfloat16
    fp8 = ml_dtypes.float8_e4m3

    def part_fold(m2d):
        # [K, M] -> [128, (K//128)*M]: row k*128+p lands at [p, k*M+m]
        kk, mm = m2d.shape
        return np.ascontiguousarray(
            m2d.reshape(kk // P, P, mm).transpose(1, 0, 2).reshape(P, -1))

    w1_dev = part_fold(t1[:, :, 0, 0].T).astype(fp8)
    # w2_dev[p, j, tap, e, m] = t2[j*128+m, e*128+p, dy, dx]
    w2_dev = np.ascontiguousarray(
        t2.reshape(W_T, P, W_T, P, 3, 3)           # j, m, e, p, dy, dx
        .transpose(3, 0, 4, 5, 2, 1)               # p, j, dy, dx, e, m
        .reshape(P, W_T * 9 * W_T * P)).astype(fp8)
    w3_dev = part_fold((t3[:, :, 0, 0] * a3[:, None]).T).astype(bf)

    cc = np.zeros((P, 8 + COUT_T), np.float32)
    cc[:, 0:2] = a1.reshape(W_T, P).T
    cc[:, 2:4] = c1.reshape(W_T, P).T
    cc[:, 4:6] = a2.reshape(W_T, P).T
    cc[:, 6:8] = c2.reshape(W_T, P).T
    cc[:, 8:] = c3.reshape(COUT_T, P).T

    const = {"w1": w1_dev, "w2": w2_dev, "w3": w3_dev,
             "cc": np.ascontiguousarray(cc)}

    x = np.asarray(x, np.float32) + c3[None, :, None, None]
    in_maps = []
    for c in range(N_CORES):
        # xd[p, bp*XB + k*NN + (r*W+cw)*2 + i] = x[c*BC+bp*2+i, k*128+p, r, cw]
        xb = x[c * BC:(c + 1) * BC].reshape(BP, IPG, CIN_T, P, H, W)
        xc = np.ascontiguousarray(
            xb.transpose(3, 0, 2, 4, 5, 1).reshape(P, BP * XB))
        # xq[p, bp*XQ + kp*2*NN + ((r*W+cw)*2+i)*2 + e] = x'[ch=(2*kp+e)*128+p]
        xq = xb.reshape(BP, IPG, KP1, 2, P, H, W)
        xq = np.ascontiguousarray(
            xq.transpose(4, 0, 2, 5, 6, 1, 3).reshape(P, BP * XQ))
        in_maps.append({"x": xc.astype(bf), "xq": xq.astype(fp8), **const})
    return in_maps


def _run(inputs, trace=False, act_func=None, **spmd_kwargs):
    nc = build(act_func)
    in_maps = _prep_host(**inputs)
    res = run_bass_kernel_spmd(nc, in_maps, list(range(N_CORES)),
                               trace=trace, **spmd_kwargs)
    outs = []
    for c in range(N_CORES):
        of = res.results[c]["out"].astype(np.float32)    # folded [P, COUT_T*BC*PIX]
        oc = of.reshape(P, COUT_T, BP, H, W, IPG)
        oc = oc.transpose(2, 5, 1, 0, 3, 4).reshape(BC, COUT, H, W)
        outs.append(oc)
    full = np.concatenate(outs, axis=0).astype(np.float32)
    return full, res


def kernel(**inputs):
    out, _ = _run(inputs)
    return out
